# revision 9
# baseline (speedup 1.0000x reference)
"""Trainium2 Bass kernel for CollapsedPBFA (collapsed Chebyshev linear attention).

Full-input contract: kernel(x, W_in, W_out) -> (B, S, D) float32.

Sharding: B x H = 2 x 16 = 32 (batch, head) pairs; each of the 8 cores owns
one batch element's 4-head block (cores 0-3 -> b=0, cores 4-7 -> b=1).
QKV projection is column-parallel per head block; the output projection is
row-parallel (each core computes a partial (S, D) product over its 256
hidden columns) and the host sums the per-core partials per batch element.

Structure (v2): the 8 s-tiles are processed as two 4-tile halves so every
elementwise op spans 4 tiles (fewer, wider instructions), with tile-pool
double buffering pipelining the halves.  Engine assignment balances the
four compute engines:
  - PE: QKV matmuls, blocked-triangular cumsum + rank-1 carries, out proj.
  - Scalar (ACT): PSUM evacuations and the square/affine feature ops.
  - Vector (DVE): TT/STT feature ops, row-sum reduces, num tree, den, outh.
  - GpSimd: the bulk Tk*v and Tq*kv multiplies for low Chebyshev orders.
Out_h transposes ride the DMA engines (dma_start_transpose) and the final
projection is DMA'd straight from PSUM (no evac copy).

Algebra (unchanged from baseline): beta is nonzero only for T_1..T_5;
beta_p is folded into the per-p triangular constants for the kv cumsum;
the den path runs as a separate 20-channel cumsum (plain tril) with beta
applied in tiny per-p STT ops; s is block-reversed within each 128-tile so
the running prefix of each chunk sits on partition 0 (rank-1 carry matmul).
"""

import sys

for _p in ("/opt/trn_rl_repo", "/root/.axon_site/_ro/trn_rl_repo"):
    if _p not in sys.path:
        sys.path.append(_p)

import numpy as np

import concourse.bacc as bacc
import concourse.bass as bass
import concourse.tile as tile
from concourse import mybir

F32 = mybir.dt.float32
BF16 = mybir.dt.bfloat16

B, S, D = 2, 1024, 1024
H, DH = 16, 64
HPC = 4                    # heads per core
EC = HPC * DH              # 256 feature cols per core side
NP = 5                     # Chebyshev orders 1..5
NS = S // 128              # 8 s-tiles
NKD = D // 128             # 8 k-tiles over d for QKV
EPS_DEN = 1e-7
INV_SQRT_D = 1.0 / 8.0     # 1/sqrt(64)
KSC = 256                  # ks channels live at kvt[...,4, 256:276]


def _beta():
    j = np.arange(6, dtype=np.float32)
    alpha = (j + 1.0) ** (-1.5)
    tail = np.flip(np.cumsum(np.flip(alpha)))
    beta = np.concatenate([np.zeros(1, np.float32), tail[1:].astype(np.float32),
                           np.zeros(5, np.float32)])
    return beta / beta.sum()          # (11,); nonzero at 1..5


def _bcast(ap, reps):
    """Broadcast a [P, ...] AP by appending a step-0 inner dim of size reps."""
    return bass.AP(tensor=ap.tensor, offset=ap.offset,
                   ap=list(ap.ap) + [[0, reps]])


def _build():
    nc = bacc.Bacc("TRN2", target_bir_lowering=False, debug=False, num_devices=8)

    XT = nc.dram_tensor("xt", [D, S], BF16, kind="ExternalInput")
    WQKVT = nc.dram_tensor("wqkvt", [D, 3 * EC], BF16, kind="ExternalInput")
    WOUTT = nc.dram_tensor("woutt", [EC, D], BF16, kind="ExternalInput")
    LTB = nc.dram_tensor("ltb", [NP, 128, 128], BF16, kind="ExternalInput")
    LTP = nc.dram_tensor("ltp", [128, 128], BF16, kind="ExternalInput")
    PART = nc.dram_tensor("part", [S, D], F32, kind="ExternalOutput")

    AX = mybir.AxisListType.X
    OP = mybir.AluOpType
    ACT = mybir.ActivationFunctionType

    beta = _beta()

    with tile.TileContext(nc) as tc:
        with (
            nc.allow_low_precision(reason="bf16 feature pipeline by design"),
            tc.tile_pool(name="persist", bufs=1) as pp,
            tc.tile_pool(name="work", bufs=2) as wp,
            tc.tile_pool(name="ps_qkv", bufs=2, space="PSUM") as ps_qkv,
            tc.tile_pool(name="ps_kv", bufs=3, space="PSUM") as ps_kv,
            tc.tile_pool(name="ps_o", bufs=1, space="PSUM") as ps_o,
        ):
            xt = pp.tile([128, NKD, S], BF16)
            wqkvt = pp.tile([128, NKD, 3 * EC], BF16)
            woutt = pp.tile([128, 2, D], BF16)
            ltb = pp.tile([128, NP, 128], BF16)
            ltp = pp.tile([128, 128], BF16)
            ones1 = pp.tile([1, 128], BF16)
            outt = pp.tile([128, 2, S], BF16)

            for k in range(NKD):
                nc.sync.dma_start(out=xt[:, k, :], in_=XT[128 * k:128 * (k + 1), :])
                nc.sync.dma_start(out=wqkvt[:, k, :], in_=WQKVT[128 * k:128 * (k + 1), :])
            for k in range(2):
                nc.sync.dma_start(out=woutt[:, k, :], in_=WOUTT[128 * k:128 * (k + 1), :])
            for p in range(NP):
                nc.sync.dma_start(out=ltb[:, p, :], in_=LTB[p])
            nc.sync.dma_start(out=ltp, in_=LTP.ap())
            nc.vector.memset(ones1, 1.0)

            # ---------------- QKV for all 8 tiles up front: PE streams densely
            # t rows per p: [q 0:256 | k 256:512 | v 512:768] (v only on p=0)
            ts = []
            for hf in range(2):
                t = wp.tile([128, 4, NP, 768], BF16, tag="t", name=f"t{hf}")
                ts.append(t)
                for jj in range(4):
                    i = 4 * hf + jj
                    si = slice(128 * i, 128 * (i + 1))
                    qkv = ps_qkv.tile([128, 768], F32, tag="qkv")
                    for k in range(NKD):
                        lhs = xt[:, k, si]
                        nc.tensor.matmul(qkv[:, 0:512], lhs, wqkvt[:, k, 0:512],
                                         start=(k == 0), stop=(k == NKD - 1))
                        nc.tensor.matmul(qkv[:, 512:768], lhs, wqkvt[:, k, 512:768],
                                         start=(k == 0), stop=(k == NKD - 1))
                    nc.scalar.copy(out=t[:, jj, 0, :], in_=qkv)

            kvt_prev = None
            for hf in range(2):
                tiles = [4 * hf + j for j in range(4)]
                t = ts[hf]

                # ---------------- Chebyshev features, wide over the half
                x1 = t[:, :, 0, 0:512]
                t2 = t[:, :, 1, 0:512]
                t3 = t[:, :, 2, 0:512]
                t4 = t[:, :, 3, 0:512]
                t5 = t[:, :, 4, 0:512]
                m2 = wp.tile([128, 4, 512], BF16, tag="sq")
                nc.scalar.activation(out=m2, in_=x1, func=ACT.Square)
                nc.scalar.activation(out=t2, in_=m2, func=ACT.Copy,
                                     scale=2.0, bias=-1.0)
                w3 = wp.tile([128, 4, 512], BF16, tag="tt")
                nc.vector.tensor_scalar(out=w3, in0=t2, scalar1=2.0, scalar2=-1.0,
                                        op0=OP.mult, op1=OP.add)
                nc.vector.tensor_tensor(out=t3, in0=x1, in1=w3, op=OP.mult)
                m4 = wp.tile([128, 4, 512], BF16, tag="sq")
                nc.scalar.activation(out=m4, in_=t2, func=ACT.Square)
                nc.scalar.activation(out=t4, in_=m4, func=ACT.Copy,
                                     scale=2.0, bias=-1.0)
                m5 = wp.tile([128, 4, 512], BF16, tag="tt")
                nc.gpsimd.tensor_mul(m5, t2, t3)
                w5 = wp.tile([128, 4, 512], BF16, tag="tt")
                nc.vector.tensor_scalar_mul(out=w5, in0=m5, scalar1=2.0)
                nc.vector.tensor_tensor(out=t5, in0=w5, in1=x1, op=OP.subtract)

                # ---------------- row-sums (per head) for den
                qs = wp.tile([128, 4, NP, HPC], BF16, tag="qs")
                ks = wp.tile([128, 4, NP, HPC], BF16, tag="ks")
                nc.vector.tensor_reduce(
                    out=qs,
                    in_=t[:, :, :, 0:256].rearrange("a j p (h d) -> a j p h d",
                                                    h=HPC),
                    axis=AX, op=OP.add)
                nc.vector.tensor_reduce(
                    out=ks,
                    in_=t[:, :, :, 256:512].rearrange("a j p (h d) -> a j p h d",
                                                      h=HPC),
                    axis=AX, op=OP.add)

                # ---------------- Tv = Tk * v  (gp: p0-2, vector: p3-4)
                tv = wp.tile([128, 4, NP, 256], BF16, tag="tv")
                vsl = t[:, :, 0, 512:768]
                for p in range(NP):
                    eng = nc.gpsimd if p < 3 else nc.vector
                    eng.tensor_mul(tv[:, :, p, :], t[:, :, p, 256:512], vsl)

                # ---------------- causal cumsum: 3 PSUM-packed chains per tile
                # kvt rows: p0..4 -> [kv 0:256]; p4 row also holds ks at 256:276
                kvt = wp.tile([128, 4, NP, 276], BF16, tag="kvt")
                for jj, i in enumerate(tiles):
                    first = (i == 0)
                    pj = jj - 1
                    # chain A: p0, p1
                    kva = ps_kv.tile([128, 512], F32, tag="kv")
                    for p in (0, 1):
                        o = kva[:, 256 * p:256 * (p + 1)]
                        nc.tensor.matmul(o, ltb[:, p, :], tv[:, jj, p, :],
                                         start=True, stop=first)
                        if not first:
                            cr = (kvt[0:1, pj, p, 0:256] if jj
                                  else kvt_prev[0:1, 3, p, 0:256])
                            nc.tensor.matmul(o, ones1, cr, start=False, stop=True)
                    nc.scalar.copy(out=kvt[:, jj, 0:2, 0:256],
                                   in_=kva.rearrange("a (p d) -> a p d", p=2))
                    # chain B: p2, p3
                    kvb = ps_kv.tile([128, 512], F32, tag="kv")
                    for p in (2, 3):
                        o = kvb[:, 256 * (p - 2):256 * (p - 1)]
                        nc.tensor.matmul(o, ltb[:, p, :], tv[:, jj, p, :],
                                         start=True, stop=first)
                        if not first:
                            cr = (kvt[0:1, pj, p, 0:256] if jj
                                  else kvt_prev[0:1, 3, p, 0:256])
                            nc.tensor.matmul(o, ones1, cr, start=False, stop=True)
                    nc.vector.tensor_scalar_add(
                        out=kvt[:, jj, 2:4, 0:256],
                        in0=kvb.rearrange("a (p d) -> a p d", p=2), scalar1=0.0)
                    # chain C: p4 + the 20 ks channels (plain tril, no beta)
                    kvc = ps_kv.tile([128, 512], F32, tag="kv")
                    nc.tensor.matmul(kvc[:, 0:256], ltb[:, 4, :], tv[:, jj, 4, :],
                                     start=True, stop=first)
                    nc.tensor.matmul(kvc[:, 256:276], ltp, ks[:, jj],
                                     start=True, stop=first)
                    if not first:
                        cr = (kvt[0:1, pj, 4, 0:276] if jj
                              else kvt_prev[0:1, 3, 4, 0:276])
                        nc.tensor.matmul(kvc[:, 0:276], ones1, cr,
                                         start=False, stop=True,
                                         skip_group_check=True)
                    nc.scalar.copy(out=kvt[:, jj, 4, 0:276], in_=kvc[:, 0:276])
                kvt_prev = kvt

                # ---------------- num: prods = Tq_p * kv_p (beta in LTB), tree
                prods = wp.tile([128, 4, NP, 256], BF16, tag="prods", bufs=1)
                nc.gpsimd.tensor_mul(prods, t[:, :, :, 0:256],
                                     kvt[:, :, :, 0:256])
                a01 = wp.tile([128, 4, 256], BF16, tag="a01")
                a23 = wp.tile([128, 4, 256], BF16, tag="a23")
                numq = wp.tile([128, 4, 256], BF16, tag="numq", bufs=1)
                nc.vector.tensor_add(a01, prods[:, :, 0, :], prods[:, :, 1, :])
                nc.vector.tensor_add(a23, prods[:, :, 2, :], prods[:, :, 3, :])
                nc.vector.tensor_add(a01, a01, prods[:, :, 4, :])
                nc.vector.tensor_tensor(out=numq, in0=a01, in1=a23, op=OP.add)

                # ---------------- den: bqs = beta_p*qs_p (scalar), one TT + reduce
                ksC = kvt[:, :, 4, 256:276].rearrange("a j (p h) -> a j p h",
                                                      p=NP)
                bqs = wp.tile([128, 4, NP, HPC], BF16, tag="bqs")
                for p in range(NP):
                    nc.scalar.activation(out=bqs[:, :, p, :], in_=qs[:, :, p, :],
                                         func=ACT.Copy, scale=float(beta[p + 1]))
                dpr = wp.tile([128, 4, NP, HPC], F32, tag="dpr")
                nc.vector.tensor_tensor(out=dpr, in0=bqs, in1=ksC, op=OP.mult)
                den4 = wp.tile([128, 4, HPC], F32, tag="den4")
                rden = wp.tile([128, 4, HPC], BF16, tag="rden")
                nc.vector.tensor_reduce(out=den4,
                                        in_=dpr.rearrange("a j p h -> a j h p"),
                                        axis=AX, op=OP.add)
                nc.vector.tensor_scalar_add(out=den4, in0=den4, scalar1=EPS_DEN)
                nc.vector.reciprocal(out=rden, in_=den4)
                outh = wp.tile([128, 4, 256], BF16, tag="outh")
                nc.vector.tensor_tensor(
                    out=outh.rearrange("a j (h d) -> a j h d", h=HPC),
                    in0=numq.rearrange("a j (h d) -> a j h d", h=HPC),
                    in1=_bcast(rden, DH), op=OP.mult)

                # ---------------- out_h transpose (DMA xbar) + out proj
                for jj, i in enumerate(tiles):
                    si = slice(128 * i, 128 * (i + 1))
                    nc.sync.dma_start_transpose(out=outt[:, :, si],
                                                in_=outh[:, jj, :])
                    outfull = wp.tile([128, D], F32, tag="outfull")
                    for n in range(2):
                        op_ps = ps_o.tile([128, 512], F32, tag="op")
                        for kt in range(2):
                            nc.tensor.matmul(op_ps, outt[:, kt, si],
                                             woutt[:, kt, 512 * n:512 * (n + 1)],
                                             start=(kt == 0), stop=(kt == 1))
                        dst = outfull[:, 512 * n:512 * (n + 1)]
                        if n == 0:
                            nc.scalar.copy(out=dst, in_=op_ps)
                        else:
                            nc.vector.tensor_scalar_add(out=dst, in0=op_ps,
                                                        scalar1=0.0)
                    nc.sync.dma_start(out=PART[si, :], in_=outfull)

    nc.compile()
    return nc


_NC = None


def _get_nc():
    global _NC
    if _NC is None:
        _NC = _build()
    return _NC


def _stage_inputs(x, W_in, W_out):
    import ml_dtypes
    bf = ml_dtypes.bfloat16
    beta = _beta()
    tri = np.tril(np.ones((128, 128), np.float32))
    ltb = np.stack([beta[p] * tri for p in range(1, 6)]).astype(bf)
    ltp = tri.astype(bf)
    in_maps = []
    for c in range(8):
        b, hb = divmod(c, 4)
        rs = slice(256 * hb, 256 * (hb + 1))
        wq = W_in[0 * D + 256 * hb:0 * D + 256 * (hb + 1)] * INV_SQRT_D
        wk = W_in[1 * D + 256 * hb:1 * D + 256 * (hb + 1)] * INV_SQRT_D
        wv = W_in[2 * D + 256 * hb:2 * D + 256 * (hb + 1)]
        wqkvt = np.ascontiguousarray(
            np.concatenate([wq, wk, wv], axis=0).T).astype(bf)
        xrev = x[b].T.reshape(D, NS, 128)[:, :, ::-1].reshape(D, S)
        in_maps.append({
            "xt": np.ascontiguousarray(xrev).astype(bf),
            "wqkvt": wqkvt,
            "woutt": np.ascontiguousarray(W_out[:, rs].T).astype(bf),
            "ltb": ltb,
            "ltp": ltp,
        })
    return in_maps


def kernel(x, W_in, W_out):
    from concourse.bass_utils import run_bass_kernel_spmd

    x = np.asarray(x, dtype=np.float32)
    W_in = np.asarray(W_in, dtype=np.float32)
    W_out = np.asarray(W_out, dtype=np.float32)
    nc = _get_nc()
    in_maps = _stage_inputs(x, W_in, W_out)
    res = run_bass_kernel_spmd(nc, in_maps, core_ids=list(range(8)))
    out = np.zeros((B, S, D), dtype=np.float32)
    for c in range(8):
        part = res.results[c]["part"].reshape(NS, 128, D)[:, ::-1, :].reshape(S, D)
        out[c // 4] += part
    return out
